# revision 48
# baseline (speedup 1.0000x reference)
"""Trainium2 Bass kernel for SoftMoE (LayerNorm + cosine routing + per-expert MLP).

Sharding: pure data-parallel over batch B=8 -> one batch element per NeuronCore.
No collectives. Each core computes its full (N, D) output slice.

v3 structure ("[n, es] orientation", transposed x read straight from the input):
  Host: mu_n = mu/||mu|| (exact fp32) -> muq = fp8(32*mu_n) [d, es] layout;
        musum = colsum(muq) [1, es] (for the mean-centering correction).
  P1/P2 per n-chunk (1-chunk software-pipeline lag for the PE):
    - DMA-TRANSPOSED reads of RAW x straight from x_h (never written -> no
      DRAM RAW hazard; Tile's DRAM-RAW tracking for transposed reads is
      unreliable, so roundtripping a freshly written tensor is not safe).
    - LN stats via ACT accum_out; xnbc = x_ln bf16; xq = fp8(32*x_ln)
      [n-part, d] resident (dispatch lhsT).
    - A[d] = sum_n x_ln (rank-1 dispatch correction) via ones-matmuls on the
      UNQUANTIZED bf16 x_ln (fp8 mean-noise in A costs 2.7e-2 rel err).
    - mean-rows via a PE transpose (for the centering correction lhsT).
    - xqT = fp8(32*x^T) on DVE; logits psum[n, es] = xqT.T @ muq (fp8
      DoubleRow, stationary reused across 4 es-chunks per LDW) + K=1 append
      of (-32*mean)x(musum); ACT Exp with per-partition scale t*r/1024
      evicts E bf16, accum_out -> sc[n]; DVE echq = (E-1)*EDS2 fp8 resident;
      E written to e_d DRAM (consumed only in P4 - far away, safe).
  P3: sd[es] = N + colsum(echq)/EDS2 via ones-matmuls; acol via PE transposes
      of the A row; dispatch siT[d, es] with A as eviction bias; per-expert
      MLP (bf16) software-pipelined into the next dispatch chunk.
  P4: DMA-TRANSPOSED read of e_d (written a whole phase earlier); DVE
      eqT = (eT-1)*EDS fp8 (quantize deferred one combine-chunk after its
      treads); out psum[n, d] = eqT.T @ soq (fp8 DR) + K=1 A2 append; ACT
      Copy scale=1/(sc*EDS*SOS) -> out.
"""

import numpy as np
from contextlib import ExitStack

import concourse.bass as bass
import concourse.tile as tile
from concourse import bacc
from concourse import mybir
from concourse.masks import make_identity

FP32 = mybir.dt.float32
BF16 = mybir.dt.bfloat16
FP8 = mybir.dt.float8e4
DR = mybir.MatmulPerfMode.DoubleRow
AF = mybir.ActivationFunctionType
ALU = mybir.AluOpType
AX = mybir.AxisListType

P = 128
LN_EPS = 1e-5
MUS = 32.0    # host-side mu_n pre-scale (fp8)
XQS = 32.0    # x pre-scale for fp8 (both layouts)
EDS2 = 32.0   # (E-1) pre-scale for the fp8 dispatch moving operand
EDS = 64.0    # (E-1) pre-scale for the fp8 combine stationary operand
SOS = 16.0    # slot_out pre-scale for the fp8 combine moving operand


def _bcast_ap(handle, p, free):
    """AP reading a 1-D DRAM tensor broadcast across p partitions."""
    return bass.AP(tensor=handle, offset=0, ap=[[0, p], [1, free]])


def build_softmoe(N, D, E, S, H, *, apply_gamma_beta=True, apply_b1=True,
                  apply_b2=True):
    assert S == P
    assert not apply_gamma_beta, "gamma/beta path not supported in v3"
    ES = E * S
    NT, KD, NE, QH = N // P, D // P, ES // P, H // P
    CN = 512
    JN = N // CN                         # n-chunks
    TPC = CN // P                        # P-tiles per n-chunk
    CE = 512
    JE = ES // CE                        # es-chunks (dispatch/MLP granularity)
    EPC = CE // P                        # experts per es-chunk
    CD = 512
    JD = D // CD                         # d-chunks

    nc = bacc.Bacc(None, target_bir_lowering=False, debug=False)

    x_h = nc.dram_tensor("x", [N, D], BF16, kind="ExternalInput")
    mu_h = nc.dram_tensor("mu", [D, ES], FP8, kind="ExternalInput")
    ms_h = nc.dram_tensor("musum", [1, ES], BF16, kind="ExternalInput")
    sc_h = nc.dram_tensor("scale", [1], FP32, kind="ExternalInput")
    w1_h = nc.dram_tensor("W1", [E, D, H], BF16, kind="ExternalInput")
    b1_h = nc.dram_tensor("b1", [E, H], FP32, kind="ExternalInput")
    w2_h = nc.dram_tensor("W2", [E, H, D], BF16, kind="ExternalInput")
    b2_h = nc.dram_tensor("b2", [E, D], FP32, kind="ExternalInput")
    out_h = nc.dram_tensor("out", [N, D], FP32, kind="ExternalOutput")

    e_d = nc.dram_tensor("e_scr", [N, ES], BF16, kind="ExternalOutput")

    with tile.TileContext(nc, pool_alloc_mode="queue") as tc, ExitStack() as ctx:
        small = ctx.enter_context(tc.tile_pool(name="small", bufs=1))

        ones_b = small.tile([P, 1], BF16, tag="ones_b")
        nc.vector.memset(ones_b, 1.0)
        ones_8 = small.tile([P, 1], FP8, tag="ones_8")
        nc.vector.memset(ones_8, 1.0)
        ones_row = small.tile([1, CE], BF16, tag="ones_row")
        nc.vector.memset(ones_row, 1.0)
        s_bc = small.tile([P, 1], FP32, tag="s_bc")
        nc.gpsimd.dma_start(out=s_bc, in_=_bcast_ap(sc_h, P, 1))
        musum_sb = small.tile([1, ES], BF16, tag="musum_sb")
        nc.gpsimd.dma_start(out=musum_sb, in_=ms_h[:, :])
        trdiv = small.tile([P, NT], FP32, tag="trdiv")    # t*r/1024 exp scale
        mcol = small.tile([P, NT], FP32, tag="mcol")      # LN means (columns)
        # -32*mean rows, one [1, N/TPC] tile per in-chunk tile index (matmul
        # lhsT requires base partition 0)
        mrow = [small.tile([1, N // TPC], BF16, tag=f"mrow{t}",
                           name=f"mrow{t}") for t in range(N // 512)]
        scv = small.tile([P, NT], FP32, tag="scv")        # sum_es E[n, :]
        scinvq = small.tile([P, NT], FP32, tag="scinvq")
        sdinv = small.tile([P, NE], FP32, tag="sdinv")
        sdcol = small.tile([P, NE], FP32, tag="sdcol")
        acol = small.tile([P, KD], FP32, tag="acol")
        A_sb = small.tile([1, D], FP32, tag="A_sb")
        nc.vector.memset(A_sb, 0.0)
        ident_b = small.tile([P, P], BF16, tag="ident_b")
        make_identity(nc, ident_b)
        ident_f = small.tile([P, P], FP32, tag="ident_f")
        make_identity(nc, ident_f)
        # accum_out slots for sc (per tile per es-chunk)
        scslots = small.tile([P, NT * JE], FP32, tag="scslots")

        # fp8 32*x_ln in [n-part, n-tile, d] layout: dispatch DoubleRow lhsT
        xqp = ctx.enter_context(tc.tile_pool(name="xq_pool", bufs=1))
        xq = xqp.tile([P, NT, D], FP8, tag="xq")
        # fp8 EDS2*(E-1) in [n-part, n-tile, es]: dispatch moving operand
        echqp = ctx.enter_context(tc.tile_pool(name="echq_pool", bufs=1))
        echq = echqp.tile([P, NT, ES], FP8, tag="echq")

        mub_ctx = ExitStack()
        mubp = mub_ctx.enter_context(tc.tile_pool(name="mub_pool", bufs=1))
        muq = mubp.tile([P, KD, ES], FP8, tag="muq")
        nc.gpsimd.dma_start(
            out=muq[:], in_=mu_h[:, :].rearrange("(k p) es -> p k es", p=P))

        # ---------------- P1 + P2: LN, x^T, logits/exp -----------------------
        with tc.tile_pool(name="p1", bufs=2) as p1, \
                tc.tile_pool(name="p1n", bufs=1) as p1n, \
                tc.tile_pool(name="p1s", bufs=8) as p1s, \
                tc.tile_pool(name="xnt_pool", bufs=2) as xntp, \
                tc.tile_pool(name="xqt_pool", bufs=2) as xqtp, \
                tc.tile_pool(name="psum2", bufs=7, space="PSUM") as psum2, \
                tc.tile_pool(name="p2b", bufs=2) as p2b:

            def logits_exp(j, xqt):
                # logits + exp per n-tile; stationary reused across es-chunks
                for t in range(TPC):
                    i = j * TPC + t
                    ps = [psum2.tile([P, CE], FP32, tag="lgps",
                                     name=f"lgps{i}_{c}", bufs=7)
                          for c in range(JE)]
                    for r in range(0, KD, 2):
                        for c in range(JE):
                            nc.tensor.matmul(ps[c][:],
                                             xqt[:, r:r + 2,
                                                 t * P:(t + 1) * P],
                                             muq[:, r:r + 2,
                                                 c * CE:(c + 1) * CE],
                                             start=(r == 0), stop=False,
                                             perf_mode=DR,
                                             skip_group_check=True)
                    for c in range(JE):
                        # mean-centering: psum += (-32*mean[n]) x musum[es]
                        nc.tensor.matmul(ps[c][:],
                                         mrow[t][0:1, j * P:(j + 1) * P],
                                         musum_sb[0:1, c * CE:(c + 1) * CE],
                                         start=False, stop=True,
                                         skip_group_check=True)
                    ebf = p2b.tile([P, ES], BF16, tag="ebf")
                    for c in range(JE):
                        nc.scalar.activation(out=ebf[:, c * CE:(c + 1) * CE],
                                             in_=ps[c][:], func=AF.Exp,
                                             scale=trdiv[:, i:i + 1],
                                             accum_out=scslots[:, i * JE + c:
                                                              i * JE + c + 1])
                        nc.vector.tensor_scalar(
                            out=echq[:, i, c * CE:(c + 1) * CE],
                            in0=ebf[:, c * CE:(c + 1) * CE],
                            scalar1=1.0, scalar2=EDS2,
                            op0=ALU.subtract, op1=ALU.mult)
                    nc.scalar.dma_start(
                        out=e_d[i * P:(i + 1) * P, :], in_=ebf[:])

            ssqv = small.tile([P, NT], FP32, tag="ssqv")
            prev = None
            for j in range(JN):
                jj = slice(j * TPC, (j + 1) * TPC)
                # transposed reads of RAW x (static input -> no RAW hazard)
                xnt = xntp.tile([P, KD, CN], BF16, tag="xnt")
                for k in range(KD):
                    eng = nc.sync if k % 2 == 0 else nc.scalar
                    eng.dma_start(
                        out=xnt[:, k, :],
                        in_=x_h[j * CN:(j + 1) * CN, k * P:(k + 1) * P],
                        transpose=True)
                xcs = []
                for t in range(TPC):
                    xct = p1.tile([P, D], BF16, tag="xc", bufs=4)
                    nc.sync.dma_start(
                        out=xct[:],
                        in_=x_h[(j * TPC + t) * P:(j * TPC + t + 1) * P, :])
                    xcs.append(xct)
                # xqT quantize on DVE (waits only the transposed reads)
                xqt = xqtp.tile([P, KD, CN], FP8, tag="xqt")
                nc.vector.tensor_scalar_mul(xqt[:], xnt[:], XQS)
                # LN stats: grouped per function (one ACT table load each),
                # then ONE [P, TPC]-wide batched small-op chain (cross-engine
                # ping-pong per tile was latency-binding the pipeline fill).
                # The stat ops' dummy outputs land on xnbc, which the real
                # x_ln write then overwrites.
                xnbc = p1n.tile([P, TPC, D], BF16, tag="xnbc")
                for t in range(TPC):
                    nc.scalar.activation(out=xnbc[:, t, :], in_=xcs[t][:, :],
                                         func=AF.Copy, scale=1.0 / float(D),
                                         accum_out=mcol[:, j * TPC + t:
                                                        j * TPC + t + 1])
                for t in range(TPC):
                    nc.scalar.activation(out=xnbc[:, t, :], in_=xcs[t][:, :],
                                         func=AF.Square,
                                         accum_out=ssqv[:, j * TPC + t:
                                                        j * TPC + t + 1])
                m2v = p1s.tile([P, TPC], FP32, tag="m2v")
                nc.vector.tensor_mul(m2v[:], mcol[:, jj], mcol[:, jj])
                varv = p1s.tile([P, TPC], FP32, tag="varv")
                nc.vector.tensor_scalar_mul(varv[:], ssqv[:, jj],
                                            1.0 / float(D))
                nc.vector.tensor_sub(varv[:], varv[:], m2v[:])
                denv = p1s.tile([P, TPC], FP32, tag="denv")
                nc.vector.tensor_scalar_add(denv[:], varv[:], LN_EPS)
                qv = p1s.tile([P, TPC], FP32, tag="qv")
                nc.scalar.activation(out=qv[:], in_=denv[:], func=AF.Sqrt)
                rv = p1s.tile([P, TPC], FP32, tag="rv")
                nc.vector.reciprocal(out=rv[:], in_=qv[:])
                rdenv = p1s.tile([P, TPC], FP32, tag="rdenv")
                nc.vector.reciprocal(out=rdenv[:], in_=denv[:])
                wv = p1s.tile([P, TPC], FP32, tag="wv")
                nc.vector.tensor_mul(wv[:], varv[:], rdenv[:])
                sq2v = p1s.tile([P, TPC], FP32, tag="sq2v")
                nc.scalar.activation(out=sq2v[:], in_=wv[:], func=AF.Sqrt,
                                     scale=float(D))
                rc2v = p1s.tile([P, TPC], FP32, tag="rc2v")
                nc.vector.reciprocal(out=rc2v[:], in_=sq2v[:])
                trv = p1s.tile([P, TPC], FP32, tag="trv")
                nc.vector.tensor_scalar(out=trv[:], in0=rc2v[:],
                                        scalar1=s_bc[:],
                                        scalar2=1.0 / 1024.0,
                                        op0=ALU.mult, op1=ALU.mult)
                nc.vector.tensor_mul(trdiv[:, jj], trv[:], rv[:])
                # x_ln and the fp8 dispatch lhsT, per tile
                for t in range(TPC):
                    i = j * TPC + t
                    nc.vector.tensor_scalar(out=xnbc[:, t, :],
                                            in0=xcs[t][:, :],
                                            scalar1=mcol[:, i:i + 1],
                                            scalar2=rv[:, t:t + 1],
                                            op0=ALU.subtract, op1=ALU.mult)
                    nc.vector.tensor_scalar_mul(xq[:, i, :], xnbc[:, t, :],
                                                XQS)

                if prev is not None:
                    logits_exp(prev[0], prev[1])
                prev = (j, xqt)

                # -32*mean rows for the centering append (PE transposes);
                # emitted after the previous chunk's logits so the PE FIFO
                # doesn't block on this chunk's stats
                paux = psum2.tile([P, CE], FP32, tag="aux", name=f"mT{j}",
                                  bufs=1)
                for t in range(TPC):
                    nc.tensor.transpose(paux[0:1, t * P:(t + 1) * P],
                                        mcol[:, j * TPC + t:j * TPC + t + 1],
                                        ident_f[:])
                for t in range(TPC):
                    nc.vector.tensor_scalar_mul(
                        mrow[t][0:1, j * P:(j + 1) * P],
                        paux[0:1, t * P:(t + 1) * P], -XQS)
                # A += colsum(x_ln) per d-chunk (bf16 rhs: clean rank-1 A)
                for dch in range(JD):
                    pA = psum2.tile([P, CE], FP32, tag="aux",
                                    name=f"pA{j}_{dch}", bufs=1)
                    for t in range(TPC):
                        nc.tensor.matmul(pA[:1, :], ones_b[:],
                                         xnbc[:, t, dch * CD:(dch + 1) * CD],
                                         start=(t == 0), stop=(t == TPC - 1),
                                         skip_group_check=True)
                    nc.vector.tensor_add(A_sb[:, dch * CD:(dch + 1) * CD],
                                         A_sb[:, dch * CD:(dch + 1) * CD],
                                         pA[:1, :])
            logits_exp(prev[0], prev[1])
            # sc = sum_es E; scinv folds the combine fp8 scales
            for i in range(NT):
                nc.vector.tensor_reduce(out=scv[:, i:i + 1],
                                        in_=scslots[:, i * JE:(i + 1) * JE],
                                        axis=AX.X, op=ALU.add)
            nc.vector.tensor_scalar_mul(scv[:], scv[:], float(EDS * SOS))
            nc.vector.reciprocal(out=scinvq[:], in_=scv[:])
        mub_ctx.close()

        # ---------------- P3: sd, acol, dispatch + pipelined MLP -------------
        # fp8 SOS*so in [s-part, expert, d] layout: combine moving operand
        soqp = ctx.enter_context(tc.tile_pool(name="soq_pool", bufs=1))
        soq = soqp.tile([P, NE, D], FP8, tag="soq")
        rows = ctx.enter_context(tc.tile_pool(name="rows", bufs=1))
        A2acc = rows.tile([1, D], FP32, tag="A2acc")
        nc.vector.memset(A2acc, 0.0)
        A2_sb = rows.tile([1, D], BF16, tag="A2_sb")
        p3_ctx = ExitStack()
        sitp = p3_ctx.enter_context(tc.tile_pool(name="sit_pool", bufs=1))
        mlp = p3_ctx.enter_context(tc.tile_pool(name="mlp", bufs=2))
        mlpw1 = p3_ctx.enter_context(tc.tile_pool(name="mlp_w1", bufs=2))
        mlpw2 = p3_ctx.enter_context(tc.tile_pool(name="mlp_w2", bufs=2))
        mlpsm = p3_ctx.enter_context(tc.tile_pool(name="mlp_sm", bufs=4))
        sobp = p3_ctx.enter_context(tc.tile_pool(name="sob", bufs=1))
        psum = p3_ctx.enter_context(
            tc.tile_pool(name="psum3", bufs=6, space="PSUM"))

        # acol[d-part, k] from the A row (16 tiny PE transposes)
        for k in range(KD):
            pac = psum.tile([P, 1], FP32, tag="pst", name=f"pac{k}", bufs=2)
            nc.tensor.transpose(pac[:], A_sb[:1, k * P:(k + 1) * P],
                                ident_f[:1, :1])
            nc.vector.tensor_copy(out=acol[:, k:k + 1], in_=pac[:])

        # sd[es] = N + colsum(echq)/EDS2 via ones-matmuls over n partitions
        sdps = [psum.tile([1, CE], FP32, tag="mmps", name=f"sdps{c}")
                for c in range(JE)]
        for r in range(NT):
            for c in range(JE):
                nc.tensor.matmul(sdps[c][:], ones_8[:],
                                 echq[:, r, c * CE:(c + 1) * CE],
                                 start=(r == 0), stop=(r == NT - 1),
                                 skip_group_check=True)
        sdrow = rows.tile([1, ES], FP32, tag="sdrow")
        for c in range(JE):
            nc.vector.tensor_scalar(out=sdrow[:, c * CE:(c + 1) * CE],
                                    in0=sdps[c][:], scalar1=1.0 / EDS2,
                                    scalar2=float(N),
                                    op0=ALU.mult, op1=ALU.add)
        for e in range(NE):
            pstn = psum.tile([P, 1], FP32, tag="pst", name=f"pstn{e}", bufs=2)
            nc.tensor.transpose(pstn[:], sdrow[:1, e * P:(e + 1) * P],
                                ident_f[:1, :1])
            nc.vector.tensor_copy(out=sdcol[:, e:e + 1], in_=pstn[:])
        nc.vector.reciprocal(out=sdinv[:], in_=sdcol[:])

        siT2 = [[sitp.tile([P, CE], BF16, tag=f"siT{par}_{d}",
                           name=f"siT{par}_{d}") for d in range(KD)]
                for par in range(2)]

        def mlp_expert(e, par):
            le = e % EPC
            KH = KD // 2
            w1e = [mlpw1.tile([P, KH, H], BF16, tag="w1e", bufs=2,
                              name=f"w1e{e}_{hh}") for hh in range(2)]
            for hh in range(2):
                nc.sync.dma_start(
                    out=w1e[hh][:],
                    in_=w1_h[e, hh * KH * P:(hh + 1) * KH * P, :]
                    .rearrange("(k p) h -> p k h", p=P))
            w2e = [mlpw2.tile([P, QH // 2, D], BF16, tag="w2e", bufs=2,
                              name=f"w2e{e}_{hh}") for hh in range(2)]
            for hh in range(2):
                nc.scalar.dma_start(
                    out=w2e[hh][:],
                    in_=w2_h[e, hh * (QH // 2) * P:(hh + 1) * (QH // 2) * P, :]
                    .rearrange("(q p) d -> p q d", p=P))
            psh = psum.tile([P, H], FP32, tag="mmps", name=f"psh{e}")
            for k in range(KD):
                nc.tensor.matmul(psh[:],
                                 siT2[par][k][:, le * P:(le + 1) * P],
                                 w1e[k // KH][:, k % KH, :],
                                 start=(k == 0),
                                 stop=(k == KD - 1 and not apply_b1))
            if apply_b1:
                pst0 = psum.tile([P, P], FP32, tag="pst", name=f"psdr{e}",
                                 bufs=2)
                nc.tensor.transpose(pst0[:1, :], sdcol[:, e:e + 1], ident_f[:])
                sdr = mlpsm.tile([1, P], BF16, tag="sdr")
                nc.vector.tensor_copy(out=sdr[:], in_=pst0[:1, :])
                b1row = mlpsm.tile([1, H], BF16, tag="b1row")
                nc.gpsimd.dma_start(out=b1row[:], in_=b1_h[e:e + 1, :])
                nc.tensor.matmul(psh[:], sdr[:], b1row[:],
                                 start=False, stop=True)
            hbf = mlp.tile([P, H], BF16, tag="hbf", bufs=2)
            nc.scalar.activation(out=hbf[:], in_=psh[:], func=AF.Gelu,
                                 scale=sdinv[:, e:e + 1])
            hT = mlp.tile([P, QH, P], BF16, tag="hT", bufs=2)
            for q in range(QH):
                pst = psum.tile([P, P], BF16, tag="pst", name=f"pst{e}_{q}",
                                bufs=2)
                nc.tensor.transpose(pst[:], hbf[:, q * P:(q + 1) * P],
                                    ident_b[:])
                nc.vector.tensor_copy(out=hT[:, q, :], in_=pst[:])
            if apply_b2:
                b2row = mlpsm.tile([1, D], BF16, tag="b2row")
                nc.gpsimd.dma_start(out=b2row[:], in_=b2_h[e:e + 1, :])
            soe = sobp.tile([P, D], BF16, tag="sob", bufs=1)
            for dch in range(JD):
                pso = psum.tile([P, CD], FP32, tag="mmps",
                                name=f"pso{e}_{dch}")
                for q in range(QH):
                    nc.tensor.matmul(
                        pso[:], hT[:, q, :],
                        w2e[q // (QH // 2)][:, q % (QH // 2),
                                            dch * CD:(dch + 1) * CD],
                        start=(q == 0), stop=(q == QH - 1 and not apply_b2))
                if apply_b2:
                    nc.tensor.matmul(
                        pso[:], ones_row[:1, :P],
                        b2row[:, dch * CD:(dch + 1) * CD],
                        start=False, stop=True)
                nc.vector.tensor_copy(
                    out=soe[:, dch * CD:(dch + 1) * CD], in_=pso[:])
            # fp8 copy for the combine + colsum(so) accumulation for A2
            nc.vector.tensor_scalar_mul(soq[:, e, :], soe[:], SOS)
            for c2 in range(JD):
                a2t = psum.tile([1, CD], FP32, tag="pst", name=f"a2t{e}_{c2}",
                                bufs=2)
                nc.tensor.matmul(a2t[:], ones_b[:],
                                 soe[:, c2 * CD:(c2 + 1) * CD])
                nc.vector.tensor_add(A2acc[:, c2 * CD:(c2 + 1) * CD],
                                     A2acc[:, c2 * CD:(c2 + 1) * CD], a2t[:])

        for c in range(JE):
            par = c % 2
            prev3 = list(range((c - 1) * EPC, c * EPC)) if c > 0 else []
            for d in range(KD):
                ps = psum.tile([P, CE], FP32, tag="mmps", name=f"sips{c}_{d}")
                for r in range(0, NT, 2):
                    nc.tensor.matmul(ps[:],
                                     xq[:, r:r + 2, d * P:(d + 1) * P],
                                     echq[:, r:r + 2, c * CE:(c + 1) * CE],
                                     start=(r == 0), stop=(r == NT - 2),
                                     perf_mode=DR)
                # siT = psum/(XQS*EDS2) + A[d]  (A bias per partition)
                nc.vector.tensor_scalar(out=siT2[par][d][:], in0=ps[:],
                                        scalar1=1.0 / float(XQS * EDS2),
                                        scalar2=acol[:, d:d + 1],
                                        op0=ALU.mult, op1=ALU.add)
                if d % 4 == 3 and prev3:
                    mlp_expert(prev3[d // 4], 1 - par)
        for e in range((JE - 1) * EPC, JE * EPC):
            mlp_expert(e, (JE - 1) % 2)
        nc.vector.tensor_scalar_mul(A2_sb[:], A2acc[:], float(EDS * SOS))
        p3_ctx.close()

        # ---------------- P4: combine (fp8 DR + rank-1 A2) -------------------
        with tc.tile_pool(name="p4e", bufs=2) as p4e, \
                tc.tile_pool(name="p4q", bufs=2) as p4q, \
                tc.tile_pool(name="psum4", bufs=8, space="PSUM") as psum4, \
                tc.tile_pool(name="p4o", bufs=2) as p4o:

            def load_eT(j):
                etT = p4e.tile([P, NE, CN], BF16, tag="etT", name=f"etT{j}")
                for k in range(NE):
                    eng = nc.sync if k % 2 == 0 else nc.scalar
                    eng.dma_start(
                        out=etT[:, k, :],
                        in_=e_d[j * CN:(j + 1) * CN, k * P:(k + 1) * P],
                        transpose=True)
                return etT

            def quant_eT(j, etT):
                eqT = p4q.tile([P, NE, CN], FP8, tag="eqT", name=f"eqT{j}")
                nc.vector.tensor_scalar(out=eqT[:], in0=etT[:], scalar1=1.0,
                                        scalar2=float(EDS),
                                        op0=ALU.subtract, op1=ALU.mult)
                return eqT

            et_next = load_eT(0)
            eq_cur = quant_eT(0, et_next)
            for j in range(JN):
                eqT = eq_cur
                if j + 1 < JN:
                    et_next = load_eT(j + 1)
                for t in range(TPC):
                    if t == 1 and j + 1 < JN:
                        # quantize the next chunk while this one streams
                        eq_cur = quant_eT(j + 1, et_next)
                    i = j * TPC + t
                    pso_ = [psum4.tile([P, CD], FP32, tag="ops",
                                       name=f"ops{i}_{dd}", bufs=8)
                            for dd in range(JD)]
                    for r in range(0, NE, 2):
                        for dch in range(JD):
                            nc.tensor.matmul(
                                pso_[dch][:],
                                eqT[:, r:r + 2, t * P:(t + 1) * P],
                                soq[:, r:r + 2, dch * CD:(dch + 1) * CD],
                                start=(r == 0), stop=False, perf_mode=DR,
                                skip_group_check=True)
                    for dch in range(JD):
                        nc.tensor.matmul(pso_[dch][:], ones_row[:1, :P],
                                         A2_sb[:1, dch * CD:(dch + 1) * CD],
                                         start=False, stop=True,
                                         skip_group_check=True)
                    outt = p4o.tile([P, D], FP32, tag="outt")
                    for dch in range(JD):
                        nc.scalar.activation(
                            out=outt[:, dch * CD:(dch + 1) * CD],
                            in_=pso_[dch][:], func=AF.Copy,
                            scale=scinvq[:, i:i + 1])
                    nc.sync.dma_start(out=out_h[i * P:(i + 1) * P, :],
                                      in_=outt[:])
    nc.compile()
    return nc


_NC_CACHE = {}


def _get_nc(N, D, E, S, H, flags):
    key = (N, D, E, S, H, flags)
    if key not in _NC_CACHE:
        _NC_CACHE[key] = build_softmoe(
            N, D, E, S, H, apply_gamma_beta=flags[0], apply_b1=flags[1],
            apply_b2=flags[2])
    return _NC_CACHE[key]


def kernel(x, gamma, beta, mu, scale, W1, b1, W2, b2):
    import ml_dtypes
    from concourse.bass_utils import run_bass_kernel_spmd

    BF = ml_dtypes.bfloat16
    F8 = ml_dtypes.float8_e4m3
    x = np.asarray(x, dtype=np.float32)
    gamma = np.ascontiguousarray(np.asarray(gamma, dtype=np.float32))
    beta = np.ascontiguousarray(np.asarray(beta, dtype=np.float32))
    mu = np.asarray(mu, dtype=np.float32)
    scale = np.ascontiguousarray(np.asarray(scale, dtype=np.float32))
    W1 = np.asarray(W1, dtype=np.float32)
    b1 = np.ascontiguousarray(np.asarray(b1, dtype=np.float32))
    W2 = np.asarray(W2, dtype=np.float32)
    b2 = np.ascontiguousarray(np.asarray(b2, dtype=np.float32))

    B, N, D = x.shape
    _, E, S = mu.shape
    H = W1.shape[2]
    n_cores = 8
    assert B == n_cores, f"kernel hardcoded for B == {n_cores}, got {B}"

    # gamma/beta must fold into the LN (this kernel supports the identity
    # case; fold non-trivial gamma/beta into x on the host would change LN
    # semantics, so assert instead)
    assert not (np.any(gamma != 1.0) or np.any(beta != 0.0)
                or np.any(scale <= 0.0)), "gamma/beta path unsupported"
    flags = (False, bool(np.any(b1 != 0.0)), bool(np.any(b2 != 0.0)))
    nc = _get_nc(N, D, E, S, H, flags)

    xb = np.ascontiguousarray(x.astype(BF))
    # host: exact mu normalization (cosine routing) + fp8 staging + colsums
    mun = mu.reshape(D, E * S)
    mun = mun / np.maximum(np.sqrt((mun * mun).sum(axis=0, keepdims=True)),
                           1e-12)
    muq = np.ascontiguousarray(
        np.clip(mun * MUS, -240.0, 240.0).astype(F8))
    musum = np.ascontiguousarray(
        muq.astype(np.float32).sum(axis=0, keepdims=True).astype(BF))
    W1b = np.ascontiguousarray(W1.astype(BF))
    W2b = np.ascontiguousarray(W2.astype(BF))

    shared = dict(mu=muq, musum=musum, scale=scale, W1=W1b, b1=b1,
                  W2=W2b, b2=b2)
    in_maps = [dict(x=xb[b], **shared) for b in range(n_cores)]
    import os
    trace = bool(os.environ.get("SOFTMOE_TRACE"))
    res = run_bass_kernel_spmd(nc, in_maps, core_ids=list(range(n_cores)),
                               trace=trace)
    global LAST_RESULT
    LAST_RESULT = res
    return np.stack([r["out"] for r in res.results], axis=0)


LAST_RESULT = None
